# revision 22
# baseline (speedup 1.0000x reference)
"""APPNP GNN kernel for 8 Trainium2 NeuronCores.

Math: the APPNP propagation is linear in the node dimension and the final
projection Wl acts on the feature dimension, so they commute:
    sigmoid(h_K @ Wl + bl) with h_K = P(h0)  ==  sigmoid(P(h0 @ Wl) + bl)
where P is the K-step propagation.  We therefore propagate a scalar per node
(z = h @ Wl) instead of a 128-dim vector — 128x less propagation work.

Device plan (per NeuronCore, nodes sharded 8 ways):
  - MLP in fp32 on the tensor engine (features pre-transposed on host),
    with layer3+Wl fused on the host (w3l = W3 @ Wl).
  - 10 propagation steps: z tables sliced 16 ways across partitions
    (replicated per GPSIMD core group), ap_gather with static indices,
    bf16 multiplicity mask, block-diag-16 ones matmul to fold slices,
    bucketed segment reduce, z' = 0.9*norm*agg + 0.1*z0, AllGather.
"""

import math
import numpy as np

N = 100000
E = 1600000
F_IN = 512
ALPHA = 0.1
K_STEPS = 10

N_NC = 8            # NeuronCores
NLOC = 12800        # node slots per NC
NPAD = N_NC * NLOC  # 102400
NCORE_LOC = NLOC // 8   # 1600 node slots per gpsimd core
SLICE = NPAD // 16      # 6400 rows per table slice
MM_TILE = 512           # node tile for MLP matmuls
CHUNK_TARGET = 1024     # gather chunk size (columns)

_cache = {}


# ----------------------------------------------------------------------------
# Host preprocessing: relabel nodes, build per-core gather slot structure.
# ----------------------------------------------------------------------------

def _preprocess(src, dst):
    deg = np.bincount(dst, minlength=N).astype(np.float32)
    norm = np.where(deg > 0, deg, np.float32(1.0)).astype(np.float32) ** np.float32(-0.5)

    # per-dst slot lists: group parallel edges (same src,dst) into one slot
    # with a multiplicity; D_n = number of distinct srcs.
    order = np.argsort(dst, kind="stable")
    dst_s = dst[order]
    src_s = src[order]
    # within each dst run, sort by src to find duplicates
    key = dst_s.astype(np.int64) * N + src_s.astype(np.int64)
    okey = np.argsort(key, kind="stable")
    dst_ss = dst_s[okey]
    src_ss = src_s[okey]
    kk = dst_ss.astype(np.int64) * N + src_ss.astype(np.int64)
    uniq_mask = np.ones(E, dtype=bool)
    uniq_mask[1:] = kk[1:] != kk[:-1]
    u_dst = dst_ss[uniq_mask]          # dst of each unique slot
    u_src = src_ss[uniq_mask]          # src of each unique slot
    # multiplicity of each unique slot
    slot_id = np.cumsum(uniq_mask) - 1
    mult = np.bincount(slot_id, minlength=len(u_dst)).astype(np.float32)
    D = np.bincount(u_dst, minlength=N)          # distinct-src degree per node
    Dp = np.maximum(2, (D + 1) // 2 * 2)         # padded to even, >= 2

    # ---- deal nodes to (NC, core) round-robin by descending Dp, and build
    # per-(NC, core) bucket-ordered node lists with equalized bucket counts.
    node_order = np.argsort(-Dp, kind="stable")  # descending Dp
    # deal across 64 (nc, core) lanes
    lanes = [[] for _ in range(64)]
    for i, n in enumerate(node_order):
        lanes[i % 64].append(n)
    # per lane, nodes are automatically bucket-sorted (descending Dp) since we
    # dealt in global sorted order.  Equalize bucket counts across ALL 64
    # lanes by padding with dummy nodes (dummy => dead slots).
    # bucket counts per lane:
    lane_counts = []   # list of dict Dp->count
    all_Dps = sorted(set(Dp[node_order].tolist()), reverse=True)
    for ln in lanes:
        c = {}
        for n in ln:
            c[int(Dp[n])] = c.get(int(Dp[n]), 0) + 1
        lane_counts.append(c)
    bucket_counts = {d: max(lc.get(d, 0) for lc in lane_counts) for d in all_Dps}
    slots_per_lane = 1 + len(all_Dps)  # not used; placeholder

    # total node slots per lane after padding
    tot_nodes = sum(bucket_counts.values())
    if tot_nodes > NCORE_LOC:
        raise RuntimeError(f"bucket-padded node count {tot_nodes} > {NCORE_LOC}")
    # remaining slots are trailing dummies in a final bucket of Dp=2
    extra_dummy = NCORE_LOC - tot_nodes
    bucket_list = [(d, bucket_counts[d]) for d in all_Dps]
    if extra_dummy:
        if bucket_list and bucket_list[-1][0] == 2:
            bucket_list[-1] = (2, bucket_list[-1][1] + extra_dummy)
        else:
            bucket_list.append((2, extra_dummy))

    # W (columns per core) is computed below after bucket-start alignment.

    # ---- global relabeling: newid = nc*NLOC + core*NCORE_LOC + pos
    perm = np.full(NPAD, -1, dtype=np.int64)     # newid -> orig (or -1 dummy)
    newid_of = np.full(N, -1, dtype=np.int64)
    # node positions per lane, in bucket order
    for lane in range(64):
        nc_i, core_i = divmod(lane, 8)
        ln = lanes[lane]
        # walk buckets, fill real nodes then dummies
        li = 0
        pos = 0
        cnt_by_d = lane_counts[lane]
        for d, cmax in bucket_list:
            creal = cnt_by_d.get(d, 0) if d != 2 else cnt_by_d.get(2, 0)
            # real nodes of this bucket are the next creal in ln
            for _ in range(creal):
                n = ln[li]; li += 1
                nid = nc_i * NLOC + core_i * NCORE_LOC + pos
                perm[nid] = n
                newid_of[n] = nid
                pos += 1
            pos += cmax - creal          # dummies
        assert li == len(ln)
        assert pos == NCORE_LOC or pos <= NCORE_LOC
    assert (newid_of >= 0).all()

    # ---- per-core slot arrays: idx (row in table), mask cells
    # slot columns are laid out bucket-major / node-major.
    u_src_new = newid_of[u_src]
    u_slice = (u_src_new // SLICE).astype(np.int64)     # 0..15
    u_row = (u_src_new % SLICE).astype(np.int64)        # table row
    u_dst_new = newid_of[u_dst]

    # column index of each slot: nodes have Dp[n] columns starting at col_of[n]
    # compute col_of per (lane, bucket order)
    col_of = np.zeros(NPAD, dtype=np.int64)     # per newid: starting column
    node_lane = np.zeros(NPAD, dtype=np.int64)
    # positions within lane determine column start: cumulative of Dp in order
    # build per-lane column starts by iterating bucket_list
    lane_col_starts = np.zeros((64, NCORE_LOC), dtype=np.int64)
    colcur = 0
    poscur = 0
    bucket_spans = []   # (col_start, col_end, d, pos_start) with 16-aligned starts
    for d, cmax in bucket_list:
        colcur = (colcur + 31) // 32 * 32      # align bucket start (32 cols => 4B-aligned idx)
        for j in range(cmax):
            lane_col_starts[:, poscur + j] = colcur + j * d
        bucket_spans.append((colcur, colcur + cmax * d, d, poscur))
        poscur += cmax
        colcur += cmax * d
    W = colcur
    Wpad = (W + 31) // 32 * 32
    # per node: lane and pos
    nid_all = np.arange(NPAD)
    lane_all = (nid_all // NLOC) * 8 + (nid_all % NLOC) // NCORE_LOC
    pos_all = nid_all % NCORE_LOC
    col_of = lane_col_starts[lane_all, pos_all]
    node_lane = lane_all

    # slot offset within node: rank of each unique slot within its dst
    # u_* arrays are sorted by (dst, src) so ranks are per-dst cumcounts
    startmask = np.ones(len(u_dst), dtype=bool)
    startmask[1:] = u_dst[1:] != u_dst[:-1]
    runstart = np.maximum.accumulate(np.where(startmask, np.arange(len(u_dst)), 0))
    rank = np.arange(len(u_dst)) - runstart

    slot_col = col_of[u_dst_new] + rank          # column within the owning core
    slot_lane = node_lane[u_dst_new]             # which (nc, core)
    slot_core = slot_lane % 8
    slot_nc = slot_lane // 8

    # build per-NC arrays
    idx_w = np.zeros((N_NC, 128, Wpad // 16), dtype=np.int16)
    mask = np.zeros((N_NC, 128, Wpad), dtype=np.float32)
    for nc_i in range(N_NC):
        sel = slot_nc == nc_i
        c = slot_core[sel]
        col = slot_col[sel]
        row = u_row[sel]
        sl = u_slice[sel]
        m = mult[sel]
        # idx wrapped: index j at (partition 16c + j%16, position j//16)
        p_idx = c * 16 + (col % 16)
        idx_w[nc_i, p_idx, col // 16] = row.astype(np.int16)
        # mask at (partition 16c + slice, column col)
        np.add.at(mask[nc_i], (c * 16 + sl, col), m)

    # chunk cut list: node-boundary cuts at %16 columns, ~CHUNK_TARGET apart.
    ends_list = []
    for (ca, cb, d, pa) in bucket_spans:
        ends_list.append(ca)
        ends_list.extend(range(ca + d, cb + 1, d))
    ends_arr = np.array(ends_list)
    valid_cuts = ends_arr[ends_arr % 32 == 0]
    cuts = [0]
    while Wpad - cuts[-1] > CHUNK_TARGET + 512:
        target = cuts[-1] + CHUNK_TARGET
        cand = valid_cuts[(valid_cuts > cuts[-1]) & (valid_cuts <= target)]
        if len(cand):
            cuts.append(int(cand[-1]))
        else:
            cand = valid_cuts[valid_cuts > cuts[-1]]
            cuts.append(int(cand[0]))
    # final chunk runs to Wpad (dead tail columns are gathered but not reduced)
    cuts.append(Wpad)
    assert max(b - a for a, b in zip(cuts, cuts[1:])) <= CHUNK_TARGET + 512, cuts

    # per chunk: list of reduce segments (col_a, col_b, D, zp_a) where
    # zp_a is the starting node position (column in z' [8, NCORE_LOC]).
    bnd = bucket_spans
    chunks = []
    for ci in range(len(cuts) - 1):
        a, b = cuts[ci], cuts[ci + 1]
        segs = []
        for (ca, cb, d, pa) in bnd:
            lo = max(a, ca)
            hi = min(b, cb)
            if lo < hi:
                assert (lo - ca) % d == 0 and (hi - ca) % d == 0, (lo, hi, ca, d)
                segs.append((lo, hi, d, pa + (lo - ca) // d))
        chunks.append((a, b, segs))

    return dict(
        norm=norm, perm=perm, newid_of=newid_of,
        idx_w=idx_w, mask=mask, W=W, Wpad=Wpad, chunks=chunks,
        bucket_list=bucket_list,
    )


# ----------------------------------------------------------------------------
# Device kernel
# ----------------------------------------------------------------------------

def _build_kernel(W, Wpad, chunks):
    from concourse import bass, mybir, bacc, tile

    f32 = mybir.dt.float32
    bf16 = mybir.dt.bfloat16
    i16 = mybir.dt.int16

    nc = bacc.Bacc("TRN2", target_bir_lowering=False, debug=False,
                   num_devices=N_NC)

    # inputs (per NC)
    xt_d = nc.dram_tensor("xt", [F_IN, NLOC], bf16, kind="ExternalInput").ap()
    w1_d = nc.dram_tensor("w1", [512, 512], bf16, kind="ExternalInput").ap()
    w2_d = nc.dram_tensor("w2", [512, 256], bf16, kind="ExternalInput").ap()
    w3l_d = nc.dram_tensor("w3l", [256, 1], bf16, kind="ExternalInput").ap()
    b1_d = nc.dram_tensor("b1c", [512, 1], f32, kind="ExternalInput").ap()
    b2_d = nc.dram_tensor("b2c", [256, 1], f32, kind="ExternalInput").ap()
    c3_d = nc.dram_tensor("c3", [1, 1], f32, kind="ExternalInput").ap()
    bl_d = nc.dram_tensor("blv", [8, 1], f32, kind="ExternalInput").ap()
    nloc_d = nc.dram_tensor("nloc", [8, NCORE_LOC], f32, kind="ExternalInput").ap()
    rnorm_d = nc.dram_tensor("rnorm", [8, NCORE_LOC], f32, kind="ExternalInput").ap()
    n09_d = nc.dram_tensor("n09", [8, NCORE_LOC], f32, kind="ExternalInput").ap()
    idx_d = nc.dram_tensor("idxw", [128, Wpad // 16], i16, kind="ExternalInput").ap()
    mask_d = nc.dram_tensor("mask", [128, Wpad], bf16, kind="ExternalInput").ap()
    ldiag_d = nc.dram_tensor("ldiag", [128, 8], f32, kind="ExternalInput").ap()
    out_d = nc.dram_tensor("out", [1, NLOC], f32, kind="ExternalOutput").ap()
    z0d = nc.dram_tensor("dbgz0", [1, NLOC], f32, kind="ExternalOutput").ap()
    zk_d = nc.dram_tensor("dbgzk", [1, NLOC], f32, kind="ExternalOutput").ap()

    NT = math.ceil(NLOC / MM_TILE)

    with tile.TileContext(nc) as tc:
        with tc.tile_pool(name="sbuf", bufs=1) as pool, \
             tc.tile_pool(name="sbw", bufs=2) as wpool, \
             tc.tile_pool(name="psum", bufs=2, space="PSUM") as psum, \
             tc.tile_pool(name="dram", bufs=1, space="DRAM") as dram:

            # ---------------- MLP ----------------
            w1_t = pool.tile([128, 4 * 512], bf16)   # 4 k-tiles side by side
            w2_t = pool.tile([128, 4 * 256], bf16)
            w3l_t = pool.tile([128, 2], bf16)
            b1_t = pool.tile([128, 4], f32)
            b2_t = pool.tile([128, 2], f32)
            c3_t = pool.tile([1, 1], f32)
            bl_t = pool.tile([8, 1], f32)
            for kt in range(4):
                nc.sync.dma_start(w1_t[:, kt * 512:(kt + 1) * 512],
                                  w1_d[kt * 128:(kt + 1) * 128, :])
                nc.sync.dma_start(w2_t[:, kt * 256:(kt + 1) * 256],
                                  w2_d[kt * 128:(kt + 1) * 128, :])
                nc.sync.dma_start(b1_t[:, kt:kt + 1],
                                  b1_d[kt * 128:(kt + 1) * 128, :])
            for kt in range(2):
                nc.sync.dma_start(w3l_t[:, kt:kt + 1],
                                  w3l_d[kt * 128:(kt + 1) * 128, :])
                nc.sync.dma_start(b2_t[:, kt:kt + 1],
                                  b2_d[kt * 128:(kt + 1) * 128, :])
            nc.sync.dma_start(c3_t[:], c3_d[:])
            nc.sync.dma_start(bl_t[:], bl_d[:])

            for t in range(NT):
                n0 = t * MM_TILE
                n1 = min(n0 + MM_TILE, NLOC)
                nw = n1 - n0
                xt_t = wpool.tile([128, 4 * MM_TILE], bf16, tag="xt")
                for kt in range(4):
                    nc.sync.dma_start(
                        xt_t[:, kt * nw:(kt + 1) * nw],
                        xt_d[kt * 128:(kt + 1) * 128, n0:n1])
                h1 = wpool.tile([128, 4 * MM_TILE], bf16, tag="h1")
                for ft in range(4):
                    p1 = psum.tile([128, MM_TILE], f32, tag="p1")
                    for kt in range(4):
                        nc.tensor.matmul(
                            out=p1[:, 0:nw],
                            lhsT=w1_t[:, kt * 512 + ft * 128: kt * 512 + ft * 128 + 128],
                            rhs=xt_t[:, kt * nw:(kt + 1) * nw],
                            start=(kt == 0), stop=(kt == 3))
                    nc.scalar.activation(
                        h1[:, ft * nw:(ft + 1) * nw], p1[:, 0:nw],
                        mybir.ActivationFunctionType.Relu,
                        bias=b1_t[:, ft:ft + 1], scale=1.0)
                h2 = wpool.tile([128, 2 * MM_TILE], bf16, tag="h2")
                for ft in range(2):
                    p2 = psum.tile([128, MM_TILE], f32, tag="p2")
                    for kt in range(4):
                        nc.tensor.matmul(
                            out=p2[:, 0:nw],
                            lhsT=w2_t[:, kt * 256 + ft * 128: kt * 256 + ft * 128 + 128],
                            rhs=h1[:, kt * nw:(kt + 1) * nw],
                            start=(kt == 0), stop=(kt == 3))
                    nc.scalar.activation(
                        h2[:, ft * nw:(ft + 1) * nw], p2[:, 0:nw],
                        mybir.ActivationFunctionType.Relu,
                        bias=b2_t[:, ft:ft + 1], scale=1.0)
                pz = psum.tile([1, MM_TILE], f32, tag="pz")
                for kt in range(2):
                    nc.tensor.matmul(
                        out=pz[:, 0:nw],
                        lhsT=w3l_t[:, kt:kt + 1],
                        rhs=h2[:, kt * nw:(kt + 1) * nw],
                        start=(kt == 0), stop=(kt == 1))
                z0sb = wpool.tile([1, MM_TILE], f32, tag="z0sb")
                nc.vector.tensor_tensor(
                    out=z0sb[:, 0:nw], in0=pz[:, 0:nw],
                    in1=c3_t[:, 0:1].to_broadcast([1, nw]),
                    op=mybir.AluOpType.add)
                nc.sync.dma_start(z0d[:, n0:n1], z0sb[:, 0:nw])

            # ---------------- propagation setup ----------------
            nloc = pool.tile([8, NCORE_LOC], f32)
            rnorm = pool.tile([8, NCORE_LOC], f32)
            n09 = pool.tile([8, NCORE_LOC], f32)
            idx_t = pool.tile([128, Wpad // 16], i16)
            ldiag = pool.tile([128, 8], f32)
            mask_t = pool.tile([128, Wpad], bf16)
            nc.sync.dma_start(nloc[:], nloc_d[:])
            nc.sync.dma_start(rnorm[:], rnorm_d[:])
            nc.sync.dma_start(n09[:], n09_d[:])
            nc.sync.dma_start(idx_t[:], idx_d[:])
            nc.sync.dma_start(ldiag[:], ldiag_d[:])
            MSPL = (Wpad // 8 + 15) // 16 * 16
            for ms in range(0, Wpad, MSPL):
                me = min(ms + MSPL, Wpad)
                nc.sync.dma_start(mask_t[:, ms:me], mask_d[:, ms:me])

            # state carried through the loop is z~ = norm * z
            t0 = pool.tile([8, NCORE_LOC], f32)
            t0in = pool.tile([8, NCORE_LOC], f32)
            nc.sync.dma_start(t0in[:], z0d[:].rearrange("o (q n) -> (o q) n", q=8))
            nc.vector.tensor_tensor(out=t0in[:], in0=t0in[:], in1=nloc[:],
                                    op=mybir.AluOpType.mult)
            nc.vector.tensor_scalar_mul(t0[:], t0in[:], ALPHA)

            cin = dram.tile([1, NLOC], f32)
            cout = dram.tile([1, NPAD], f32)
            tables = pool.tile([128, SLICE], f32)
            zp = pool.tile([8, NCORE_LOC], f32)
            dummy_tab = pool.tile([128, 8], f32)
            dummy_out = pool.tile([128, 8], f32)
            dummy_idx = pool.tile([128, 1], i16)
            nc.vector.memset(dummy_tab[:], 0)
            nc.vector.memset(dummy_idx[:], 0)

            nc.sync.dma_start(cin[:].rearrange("o (q n) -> (o q) n", q=8), t0in[:])
            for step in range(K_STEPS):
              with nc.named_scope(f"step{step}"):
                  # AllGather current z~, rebuild tables (already normalized)
                  nc.gpsimd.collective_compute(
                      "AllGather", mybir.AluOpType.bypass,
                      replica_groups=[list(range(N_NC))],
                      ins=[cin.opt()], outs=[cout.opt()])
                  # dummy gather: force the Q7 ap_gather library (re)load to
                  # overlap with the collective + table DMAs
                  nc.gpsimd.ap_gather(
                      dummy_out[:], dummy_tab[:], dummy_idx[:],
                      channels=128, num_elems=8, d=1, num_idxs=4)
                  for g in range(8):
                      for hh in range(2):
                          eng = nc.sync if (g + hh) % 2 == 0 else nc.scalar
                          eng.dma_start(
                              tables[g * 16 + hh * 8:g * 16 + (hh + 1) * 8, :],
                              cout[:, hh * (NPAD // 2):(hh + 1) * (NPAD // 2)]
                              .rearrange("o (p f) -> (o p) f", p=8))

                  # chunked gather / mask / fold / reduce
                  for (ca, cb, segs) in chunks:
                      cw = cb - ca
                      g_t = wpool.tile([128, cw], f32, tag="g", bufs=4)
                      nc.gpsimd.ap_gather(
                          g_t[:], tables[:], idx_t[:, ca // 16: cb // 16],
                          channels=128, num_elems=SLICE, d=1, num_idxs=cw)
                      gm = g_t
                      nc.vector.tensor_tensor(
                          out=gm[:], in0=g_t[:], in1=mask_t[:, ca:cb],
                          op=mybir.AluOpType.mult)
                      gs = wpool.tile([8, cw], f32, tag="h1")
                      for bb in range(0, cw, 512):
                          be = min(bb + 512, cw)
                          pp = psum.tile([8, 512], f32, tag="pp")
                          nc.tensor.matmul(
                              out=pp[:, 0:be - bb], lhsT=ldiag[:],
                              rhs=gm[:, bb:be], start=True, stop=True)
                          nc.vector.tensor_copy(out=gs[:, bb:be], in_=pp[:, 0:be - bb])
                      for (sa, sb, d, pa) in segs:
                          nseg = (sb - sa) // d
                          nc.vector.tensor_reduce(
                              out=zp[:, pa:pa + nseg],
                              in_=gs[:, sa - ca:sb - ca].rearrange(
                                  "p (n d) -> p n d", d=d),
                              axis=mybir.AxisListType.X, op=mybir.AluOpType.add)

                  # z~' = agg * (0.9*norm^2) + 0.1*z~0
                  nc.vector.tensor_tensor(out=zp[:], in0=zp[:], in1=n09[:],
                                          op=mybir.AluOpType.mult)
                  nc.vector.tensor_tensor(out=zp[:], in0=zp[:], in1=t0[:],
                                          op=mybir.AluOpType.add)
                  if step < K_STEPS - 1:
                      nc.sync.dma_start(
                          cin[:].rearrange("o (q n) -> (o q) n", q=8), zp[:])

            # ---------------- output ----------------
            nc.sync.dma_start(zk_d[:].rearrange("o (q n) -> (o q) n", q=8), zp[:])
            # z_K = z~_K / norm, then sigmoid(z_K + bl)
            nc.vector.tensor_tensor(out=zp[:], in0=zp[:], in1=rnorm[:],
                                    op=mybir.AluOpType.mult)
            o_t = pool.tile([8, NCORE_LOC], f32)
            nc.scalar.activation(o_t[:], zp[:],
                                 mybir.ActivationFunctionType.Sigmoid,
                                 bias=bl_t[:, 0:1], scale=1.0)
            nc.sync.dma_start(out_d[:].rearrange("o (q n) -> (o q) n", q=8), o_t[:])

    nc.compile()
    return nc


# ----------------------------------------------------------------------------
# Entry point
# ----------------------------------------------------------------------------

def kernel(features, src, dst, W1, b1, W2, b2, W3, b3, Wl, bl,
           _want_trace=False):
    import ml_dtypes
    from concourse.bass_utils import run_bass_kernel_spmd

    pre = _preprocess(np.asarray(src), np.asarray(dst))
    norm = pre["norm"]; perm = pre["perm"]
    idx_w = pre["idx_w"]; mask = pre["mask"]
    W = pre["W"]; Wpad = pre["Wpad"]; chunks = pre["chunks"]

    key = (W, Wpad, tuple((a, b, tuple(s)) for a, b, s in chunks))
    if key not in _cache:
        _cache.clear()
        _cache[key] = _build_kernel(W, Wpad, chunks)
    nc = _cache[key]

    features = np.asarray(features, dtype=np.float32)
    W1 = np.asarray(W1, np.float32); W2 = np.asarray(W2, np.float32)
    W3 = np.asarray(W3, np.float32); Wl = np.asarray(Wl, np.float32)
    b1 = np.asarray(b1, np.float32); b2 = np.asarray(b2, np.float32)
    b3 = np.asarray(b3, np.float32); bl = np.asarray(bl, np.float32)

    w3l = (W3 @ Wl).astype(np.float32)              # [256, 1]
    c3 = (b3 @ Wl).astype(np.float32).reshape(1, 1)  # scalar

    ldiag = np.zeros((128, 8), np.float32)
    for p in range(128):
        ldiag[p, p // 16] = 1.0

    # per-NC input maps
    in_maps = []
    for nc_i in range(N_NC):
        ids = perm[nc_i * NLOC:(nc_i + 1) * NLOC]      # orig ids or -1
        xt = np.zeros((F_IN, NLOC), ml_dtypes.bfloat16)
        real = ids >= 0
        xt[:, real] = features[ids[real]].T.astype(ml_dtypes.bfloat16)
        nrm = np.ones(NLOC, np.float32)
        nrm[real] = norm[ids[real]]
        n09 = (np.float32(1.0 - ALPHA) * nrm * nrm).reshape(8, NCORE_LOC)
        in_maps.append({
            "xt": xt,
            "w1": W1.astype(ml_dtypes.bfloat16),
            "w2": W2.astype(ml_dtypes.bfloat16),
            "w3l": w3l.astype(ml_dtypes.bfloat16),
            "b1c": b1.reshape(512, 1), "b2c": b2.reshape(256, 1),
            "c3": c3, "blv": np.full((8, 1), bl[0], np.float32),
            "nloc": nrm.reshape(8, NCORE_LOC).copy(),
            "rnorm": (np.float32(1.0) / nrm).reshape(8, NCORE_LOC).copy(),
            "n09": n09,
            "idxw": idx_w[nc_i],
            "mask": mask[nc_i].astype(ml_dtypes.bfloat16),
            "ldiag": ldiag,
        })

    res = run_bass_kernel_spmd(nc, in_maps, core_ids=list(range(N_NC)),
                               trace=_want_trace)

    out = np.zeros((N, 1), np.float32)
    for nc_i in range(N_NC):
        ids = perm[nc_i * NLOC:(nc_i + 1) * NLOC]
        real = ids >= 0
        vals = res.results[nc_i]["out"].reshape(NLOC)
        out[ids[real], 0] = vals[real]
    if _want_trace:
        return out, res
    return out

